# revision 2
# baseline (speedup 1.0000x reference)
"""Trainium2 Bass kernel v2 for nn_BoxesFromMasks (per-frame segment bboxes).

Per core (TL frames), per frame, per 128-row chunk:
  - DMA seg chunk [128, 2048] i32 (8 queue-parallel sub-DMAs)
  - one-hot bit planes via the fp32-exponent trick (lo: bit 31-s for s<32,
    hi: bit s-32 for s>=32); lo-build alternates ACT/Pool, hi-build Pool,
    truncating f32->u32 cast of both planes on ACT.
  - DVE: one TT or-accumulate into acc[P, 2, 2048] (chunk 0 builds straight
    into acc), and row-mask fold (5 TT-tree levels + TR) -> rmask.
Column side per frame: DMA-transpose acc as u16 -> accT[P, 2, 32, 128],
  DVE OR-fold over the 128 source rows + halfword merge -> colm[q, pl, t]
  (q parity = halfword h; column c = 64*t + q//2).
Extraction (once): broadcast-AP bit-extract, TT-mult value select with
  negated-min tables (all folds become MAX), free-axis folds, one u16
  transpose for the partition fold, parity-aware fold for the column stats,
  DRAM bounce to land stats on partitions p = n*TL + f.
"""

import numpy as np

_T, _H, _W, _N = 16, 1024, 2048, 64
_NCORES = 8

_BUILD_CACHE = {}

_BIG = 32767


def _register_or_op():
    """Fused DVE op: out = in0 | in1; accum_out = OR-fold(out).

    Lets one pass build the half-ORed plane and its full row-reduction:
    reads 2 elems/cycle via both ports instead of the 1/cycle of a plain
    tensor_reduce over the full plane.
    """
    import numpy as np
    import concourse.dve_ops as dve_ops
    from concourse.dve_spec import Spec, Src0, Src1, Bin, AluOp, Zero, lower
    from concourse.dve_table_gen import dve_ver_for, DveOpSpec

    for o in dve_ops.OPS:
        if o.name == "ANT_OR_OR":
            return o

    def _ref(in0, in1, c0, c1, c2):
        b = np.bitwise_or(in0, in1)
        return b, np.bitwise_or.reduce(
            b.reshape(b.shape[0], -1), axis=-1, keepdims=True)

    body = Bin(AluOp.BITWISE_OR, Src0, Src1)
    spec = Spec(body=body, accum=AluOp.BITWISE_OR, accum_init=Zero,
                reference=_ref)
    opcode = dve_ops._CUSTOM_DVE_ROW_BASE + len(dve_ops.OPS)
    shas = {}
    for ver in ("v3", "v4"):
        try:
            shas[ver] = DveOpSpec(name="ANT_OR_OR", opcode=opcode,
                                  uops=lower(spec, ver=ver),
                                  rd1_en=True).sha(ver)
        except Exception:
            pass
    op = dve_ops.DveOp("ANT_OR_OR", spec, subdim=False, uops_sha=shas)
    dve_ops.OPS.append(op)
    dve_ops.CUSTOM_DVE_SPECS[op.name] = spec
    dve_ops._SUB_OPCODE_FOR_NAME[op.name] = opcode
    return op


def _build_tables(TL, H, W):
    P = 128
    CH = H // P
    p = np.arange(P)
    ch = np.arange(CH)
    y = (128 * ch[None, :] + p[:, None]).astype(np.int64)       # [P, CH]
    t = np.arange(32)
    x = (64 * t[None, :] + (p[:, None] // 2)).astype(np.int64)  # [P, 32]
    j32 = np.tile(np.arange(32), 2)                             # [64]
    return {
        "ymBn": (_BIG - y).astype(np.int32),
        "yp1": (y + 1).astype(np.int32),
        "xmBn": (_BIG - x).astype(np.int16),
        "xp1": (x + 1).astype(np.int16),
        "sp32": np.broadcast_to(j32, (P, 64)).astype(np.int32).copy(),
        "sp16": np.broadcast_to(np.arange(16), (P, 16)).astype(np.int16).copy(),
    }


def _build_program(TL, H, W, reps=1, dbg=False):
    from contextlib import ExitStack

    import concourse.bass as bass
    import concourse.tile as tile
    import concourse.mybir as mybir
    from concourse.alu_op_type import AluOpType as Op

    f32 = mybir.dt.float32
    i32 = mybir.dt.int32
    u32 = mybir.dt.uint32
    u16 = mybir.dt.uint16
    i16 = mybir.dt.int16
    Copy = mybir.ActivationFunctionType.Copy
    X = mybir.AxisListType.X

    P = 128
    CH = H // P                 # 8 row chunks per frame
    NT = 32                     # u16-word blocks per plane after transpose
    FC = TL * CH

    tables = _build_tables(TL, H, W)

    nc = bass.Bass()
    seg_in = nc.dram_tensor("seg", [TL, H, W], i32, kind="ExternalInput")
    boxes_out = nc.dram_tensor("boxes", [TL, 64, 4], f32, kind="ExternalOutput")

    def _dt(a):
        return {np.dtype(np.int32): i32, np.dtype(np.int16): i16,
                np.dtype(np.float32): f32}[a.dtype]

    d_tab = {
        n: nc.dram_tensor(n, list(tables[n].shape), _dt(tables[n]),
                          kind="ExternalInput")
        for n in tables
    }

    if dbg:
        dbg_rmask = nc.dram_tensor("dbg_rmask", [P, 2, TL, CH], i32,
                                   kind="ExternalOutput")
        dbg_colm = nc.dram_tensor("dbg_colm", [P, TL, 2, NT], i16,
                                  kind="ExternalOutput")
        dbg_S = nc.dram_tensor("dbg_S", [P, 384], i16, kind="ExternalOutput")
        dbg_ST = nc.dram_tensor("dbg_ST", [P, 3, 128], i16,
                                kind="ExternalOutput")
        dbg_xst = nc.dram_tensor("dbg_xst", [P, 32, 2], i16,
                                 kind="ExternalOutput")

    with tile.TileContext(nc) as tc, ExitStack() as ctx:
        constp = ctx.enter_context(tc.tile_pool(name="consts", bufs=1))
        segp = ctx.enter_context(tc.tile_pool(name="segp", bufs=3))
        ep = ctx.enter_context(tc.tile_pool(name="ep", bufs=3))
        accp = ctx.enter_context(tc.tile_pool(name="accp", bufs=2))
        accTp = ctx.enter_context(tc.tile_pool(name="accTp", bufs=2))
        scrp = ctx.enter_context(tc.tile_pool(name="scrp", bufs=2))
        maskp = ctx.enter_context(tc.tile_pool(name="maskp", bufs=1))
        xp = ctx.enter_context(tc.tile_pool(name="xp", bufs=1))
        smallp = ctx.enter_context(tc.tile_pool(name="smallp", bufs=1))

        c_tab = {}
        for n in tables:
            c_tab[n] = constp.tile([P, tables[n].shape[1]], _dt(tables[n]),
                                   name=f"tab_{n}")
            nc.sync.dma_start(c_tab[n][:], d_tab[n][:])

        for _rep in range(reps):
            rmask = maskp.tile([P, 2, TL, CH], i32, tag="rmask")
            colm = maskp.tile([P, TL, 2, NT], i16, tag="colm")

            # ================= main loop =================
            for f in range(TL):
                acc = accp.tile([P, 2, W], i32, tag="acc")
                for c in range(CH):
                    s = segp.tile([P, W], i32, tag="seg")
                    for k in range(2):
                        nc.sync.dma_start(
                            s[64 * k:64 * (k + 1), :],
                            seg_in[f, c * P + 64 * k:c * P + 64 * (k + 1), :])

                    tgt = acc if c == 0 else ep.tile([P, 2, W], i32, tag="e")
                    # lo plane: int bits of fp32 2^(31-s) = (158-s)<<23
                    if c % 2 == 0:
                        nc.scalar.activation(tgt[:, 0, :], s[:], Copy,
                                             bias=1325400064.0,
                                             scale=-8388608.0)
                    else:
                        nc.gpsimd.tensor_scalar(tgt[:, 0, :], s[:], -8388608,
                                                1325400064, Op.mult, Op.add)
                    # hi plane: int bits of fp32 2^(s-32) = (95+s)<<23
                    nc.gpsimd.tensor_scalar(tgt[:, 1, :], s[:], 8388608,
                                            796917760, Op.mult, Op.add)
                    # truncating cast f32 -> u32 (out-of-range ids -> 0)
                    nc.scalar.activation(tgt[:].bitcast(u32),
                                         tgt[:].bitcast(f32), Copy)

                    if c > 0:
                        nc.vector.tensor_tensor(acc[:].bitcast(u32),
                                                tgt[:].bitcast(u32),
                                                acc[:].bitcast(u32),
                                                Op.bitwise_or)

                    # row masks: single OR-reduce over the 2048 columns
                    nc.vector.tensor_reduce(rmask[:, :, f, c:c + 1].bitcast(u32),
                                            tgt[:].bitcast(u32), axis=X,
                                            op=Op.bitwise_or)

                # ---- column masks: transpose acc (u16) + OR over rows ----
                accT = accTp.tile([P, 2, NT, 128], u16, tag="accT")
                for pl in range(2):
                    a16 = acc[:, pl, :].bitcast(u16)   # [P, 4096]
                    for k in range(4):
                        nc.sync.dma_start(accT[:, pl, 8 * k:8 * (k + 1), :],
                                          a16[:, 1024 * k:1024 * (k + 1)],
                                          transpose=True)
                aT = accT[:].bitcast(u32)              # [P, 2, NT, 64]
                redT = smallp.tile([P, 2, NT], u32, tag="redT")
                nc.vector.tensor_reduce(redT[:].unsqueeze(3), aT[:], axis=X,
                                        op=Op.bitwise_or)
                # merge u16 halves (even/odd source rows), keep low u16
                hv = smallp.tile([P, 2, NT], u32, tag="hv")
                nc.vector.tensor_scalar(hv[:], redT[:], 16, None,
                                        Op.logical_shift_right)
                nc.vector.tensor_tensor(hv[:], hv[:], redT[:],
                                        Op.bitwise_or)
                hv16 = hv[:].bitcast(u16).rearrange(
                    "p a (b two) -> p a b two", two=2)
                nc.vector.tensor_copy(colm[:, f, :, :].bitcast(u16),
                                      hv16[:, :, :, 0])

            # ================= extraction =================
            if dbg:
                nc.sync.dma_start(dbg_rmask[:], rmask[:])
                nc.sync.dma_start(dbg_colm[:], colm[:])

            S = smallp.tile([P, 3, 128], i16, tag="S")

            # ---- row side: E = (rmask >> sp) & 1, dims [pl, sp, (f ch)] ----
            E32 = xp.tile([P, 2, 32, TL, CH], i32, tag="E32")
            rm_b = (rmask[:].rearrange("p a b c -> p a (b c)")
                    .unsqueeze(2).broadcast_to([P, 2, 32, FC]))
            sp_b = (c_tab["sp32"][:].rearrange("p (a b) -> p a b", a=2)
                    .unsqueeze(3).broadcast_to([P, 2, 32, FC]))
            E32f = E32[:].rearrange("p a b c d -> p a b (c d)").bitcast(u32)
            nc.vector.tensor_tensor(E32f, rm_b.bitcast(u32), sp_b.bitcast(u32),
                                    Op.logical_shift_right)
            e32flat = E32[:].rearrange("p a b c d -> p (a b c d)").bitcast(u32)
            nc.vector.tensor_scalar(e32flat, e32flat, 1, None, Op.bitwise_and)

            # select values (i16); min side pre-negated -> MAX folds
            vr = xp.tile([P, 2, 64, TL, CH], i16, tag="vr")   # [k, (pl sp), f, ch]
            E32_3 = E32[:].rearrange("p a b c d -> p (a b) c d")
            ymBn_b = (c_tab["ymBn"][:].unsqueeze(1).unsqueeze(1)
                      .broadcast_to([P, 64, TL, CH]))
            yp1_b = (c_tab["yp1"][:].unsqueeze(1).unsqueeze(1)
                     .broadcast_to([P, 64, TL, CH]))
            nc.vector.tensor_tensor(vr[:, 0], E32_3, ymBn_b, Op.mult)
            nc.vector.tensor_tensor(vr[:, 1], E32_3, yp1_b, Op.mult)
            vr0 = vr[:, 0].rearrange("p a b c -> p (a b c)")
            nc.vector.tensor_scalar(vr0, vr0, -_BIG, None, Op.add)

            # fold over ch (8 -> 1): 3 MAX levels, both k at once
            vrf = vr[:].rearrange("p k a b c -> p (k a b) c")
            w = CH // 2
            while w >= 1:
                nc.vector.tensor_tensor(vrf[:, :, 0:w], vrf[:, :, 0:w],
                                        vrf[:, :, w:2 * w], Op.max)
                w //= 2
            # S rows 0/1: col idx = (pl sp)*TL + f = n*TL + f
            nc.vector.tensor_copy(
                S[:, 0, :], vr[:, 0].rearrange("p a b c -> p (a b) c")[:, :, 0])
            nc.vector.tensor_copy(
                S[:, 1, :], vr[:, 1].rearrange("p a b c -> p (a b) c")[:, :, 0])

            # ---- col side: E = (colm >> sp16) & 1, dims [(f pl), sp16, t] --
            E16 = xp.tile([P, TL * 2, 16, NT], i16, tag="E16")
            cm_b = (colm[:].rearrange("p a b c -> p (a b) c")
                    .unsqueeze(2).broadcast_to([P, TL * 2, 16, NT]))
            s16_b = (c_tab["sp16"][:].unsqueeze(1).unsqueeze(3)
                     .broadcast_to([P, TL * 2, 16, NT]))
            nc.vector.tensor_tensor(E16[:].bitcast(u16), cm_b.bitcast(u16),
                                    s16_b.bitcast(u16),
                                    Op.logical_shift_right)
            e16flat = E16[:].rearrange("p a b c -> p (a b c)").bitcast(u16)
            nc.vector.tensor_scalar(e16flat, e16flat, 1, None, Op.bitwise_and)

            vc = xp.tile([P, 2, TL * 2 * 16, NT], i16, tag="vc")
            E16_3 = E16[:].rearrange("p a b c -> p (a b) c")
            xmBn_b = (c_tab["xmBn"][:].unsqueeze(1)
                      .broadcast_to([P, TL * 2 * 16, NT]))
            xp1_b = (c_tab["xp1"][:].unsqueeze(1)
                     .broadcast_to([P, TL * 2 * 16, NT]))
            nc.vector.tensor_tensor(vc[:, 0], E16_3, xmBn_b, Op.mult)
            nc.vector.tensor_tensor(vc[:, 1], E16_3, xp1_b, Op.mult)
            vc0 = vc[:, 0].rearrange("p a b -> p (a b)")
            nc.vector.tensor_scalar(vc0, vc0, -_BIG, None, Op.add)

            # fold over t (32 -> 1): 5 MAX levels, both k at once
            vcf = vc[:].rearrange("p k a b -> p (k a) b")
            w = NT // 2
            while w >= 1:
                nc.vector.tensor_tensor(vcf[:, :, 0:w], vcf[:, :, 0:w],
                                        vcf[:, :, w:2 * w], Op.max)
                w //= 2
            # S row 2: [-xmin stats | xmax stats], idx = f*32 + pl*16 + sp16
            nc.vector.tensor_copy(S[:, 2, 0:64], vc[:, 0, :, 0])
            nc.vector.tensor_copy(S[:, 2, 64:128], vc[:, 1, :, 0])

            if dbg:
                nc.sync.dma_start(dbg_S[:],
                                  S[:].rearrange("p a b -> p (a b)"))

            # ---- partition fold via one u16 transpose ----
            ST = smallp.tile([P, 3, 128], i16, tag="ST")
            nc.sync.dma_start(ST[:], S[:].rearrange("p a b -> p (a b)"),
                              transpose=True)
            # rows 0/1: fold over all 128 source partitions
            yst = smallp.tile([P, 2, 1], i16, tag="yst")
            nc.vector.tensor_reduce(yst[:], ST[:, 0:2, :], axis=X, op=Op.max)
            # row 2: parity-separated fold (64 pairs) -> [q, h]
            ST2 = ST[:, 2, :].rearrange("p (a two) -> p a two", two=2)
            xst = smallp.tile([P, 32, 2], i16, tag="xst")
            nc.vector.tensor_tensor(xst[:], ST2[:, 0:32, :], ST2[:, 32:64, :],
                                    Op.max)
            w = 16
            while w >= 1:
                nc.vector.tensor_tensor(xst[:, 0:w, :], xst[:, 0:w, :],
                                        xst[:, w:2 * w, :], Op.max)
                w //= 2
            # xst[q, 0, h]: q = k*64 + (f*32 + pl*16 + sp16)
            if dbg:
                nc.sync.dma_start(dbg_ST[:], ST[:])
                nc.sync.dma_start(dbg_xst[:], xst[:])

            # ---- finalize in-layout; output DMAs do the permutation ----
            # row side: yst[p = n*TL + f, {0: -ymin, 1: ymax+1}]
            YF = smallp.tile([P, 2], f32, tag="YF")
            yfix = smallp.tile([P, 2], f32, tag="yfix")
            nc.vector.tensor_copy(YF[:], yst[:, :, 0])
            nc.vector.tensor_scalar(YF[:, 0:1], YF[:, 0:1], -1.0, 0.0,
                                    Op.mult, Op.add)
            nc.vector.tensor_scalar(YF[:, 1:2], YF[:, 1:2], 1.0, 0.0,
                                    Op.subtract, Op.add)
            nc.vector.tensor_scalar(yfix[:, 0:1], YF[:, 0:1], 32767.0,
                                    2147450880.0, Op.is_equal, Op.mult)
            nc.vector.tensor_scalar(yfix[:, 1:2], YF[:, 1:2], -1.0,
                                    -2147483647.0, Op.is_equal, Op.mult)
            nc.vector.tensor_tensor(YF[:], YF[:], yfix[:], Op.add)
            # boxes[f, n, k] for k in {1, 3} <- YF[n*TL + f, k2]
            ydst = boxes_out[:].rearrange("f n (k2 two) -> n f k2 two",
                                          k2=2, two=2)[:, :, :, 1]
            ysrc = YF[:].rearrange("(n f) k -> n f k", n=64, f=TL)
            for fi in range(TL):
                nc.sync.dma_start(ydst[:, fi], ysrc[:, fi])

            # col side: xst[q = k*64 + f*32 + pl*16 + sp, 0, h]
            XF = smallp.tile([P, 2], f32, tag="XF")
            xfix = smallp.tile([P, 2], f32, tag="xfix")
            nc.vector.tensor_copy(XF[:], xst[:, 0, :])
            nc.vector.tensor_scalar(XF[0:64, :], XF[0:64, :], -1.0, 0.0,
                                    Op.mult, Op.add)
            nc.vector.tensor_scalar(XF[64:128, :], XF[64:128, :], 1.0, 0.0,
                                    Op.subtract, Op.add)
            nc.vector.tensor_scalar(xfix[0:64, :], XF[0:64, :], 32767.0,
                                    2147450880.0, Op.is_equal, Op.mult)
            nc.vector.tensor_scalar(xfix[64:128, :], XF[64:128, :], -1.0,
                                    -2147483647.0, Op.is_equal, Op.mult)
            nc.vector.tensor_tensor(XF[:], XF[:], xfix[:], Op.add)
            # boxes[f, 32*pl+16*h+sp, k] for k in {0, 2}
            xdst_all = boxes_out[:].rearrange(
                "f (pl h sp) k -> k f pl sp h", pl=2, h=2, sp=16)
            for k in range(2):
                for fi in range(TL):
                    base = 64 * k + 32 * fi
                    src_ap = XF[base:base + 32, :].rearrange(
                        "(pl sp) h -> pl sp h", pl=2, sp=16)
                    for pl in range(2):
                        nc.sync.dma_start(xdst_all[2 * k, fi, pl],
                                          src_ap[pl])

    nc.finalize()
    _split_excess_waits(nc, mybir)
    return nc, tables


def _split_excess_waits(nc, mybir):
    """Hoist extra sem waits onto preceding NoOps (walrus encoding limit)."""
    n_split = 0
    for f in nc.m.functions:
        for bb in f.blocks:
            newl = []
            for ins in bb.instructions:
                si = ins.sync_info
                max_waits = 1
                if si and si.on_wait and len(si.on_wait) > max_waits:
                    waits = list(si.on_wait)
                    for j, w in enumerate(waits[max_waits:]):
                        nop = mybir.InstNoOp(
                            name=f"{ins.name}-w{j}", ins=[], outs=[],
                            engine=ins.engine,
                            sync_info=mybir.SyncInfo(on_wait=[w],
                                                     on_update=[]))
                        newl.append(nop)
                        n_split += 1
                    ins.sync_info = mybir.SyncInfo(on_wait=waits[:max_waits],
                                                   on_update=si.on_update)
                newl.append(ins)
            bb.instructions = newl
    return n_split


def _get_program(TL, H, W, reps=1, dbg=False):
    key = (TL, H, W, reps, dbg)
    if key not in _BUILD_CACHE:
        _BUILD_CACHE[key] = _build_program(TL, H, W, reps=reps, dbg=dbg)
    return _BUILD_CACHE[key]


def kernel(segmentation, num_instances=None, **_ignored):
    from concourse.bass_utils import run_bass_kernel_spmd

    seg = np.asarray(segmentation)
    T, H, W = seg.shape
    assert T % _NCORES == 0
    TL = T // _NCORES
    nc, tables = _get_program(TL, H, W)

    seg = np.ascontiguousarray(seg, dtype=np.int32)
    in_maps = [{"seg": seg[i * TL:(i + 1) * TL], **tables}
               for i in range(_NCORES)]
    res = run_bass_kernel_spmd(nc, in_maps, list(range(_NCORES)))
    out = np.concatenate([res.results[i]["boxes"] for i in range(_NCORES)],
                         axis=0)
    return out.astype(np.float32)


# revision 3
# speedup vs baseline: 37.1908x; 37.1908x over previous
"""Trainium2 Bass kernel v2 for nn_BoxesFromMasks (per-frame segment bboxes).

Per core (TL frames), per frame, per 128-row chunk:
  - DMA seg chunk [128, 2048] i32 (8 queue-parallel sub-DMAs)
  - one-hot bit planes via the fp32-exponent trick (lo: bit 31-s for s<32,
    hi: bit s-32 for s>=32); lo-build alternates ACT/Pool, hi-build Pool,
    truncating f32->u32 cast of both planes on ACT.
  - DVE: one TT or-accumulate into acc[P, 2, 2048] (chunk 0 builds straight
    into acc), and row-mask fold (5 TT-tree levels + TR) -> rmask.
Column side per frame: DMA-transpose acc as u16 -> accT[P, 2, 32, 128],
  DVE OR-fold over the 128 source rows + halfword merge -> colm[q, pl, t]
  (q parity = halfword h; column c = 64*t + q//2).
Extraction (once): broadcast-AP bit-extract, TT-mult value select with
  negated-min tables (all folds become MAX), free-axis folds, one u16
  transpose for the partition fold, parity-aware fold for the column stats,
  DRAM bounce to land stats on partitions p = n*TL + f.
"""

import numpy as np

_T, _H, _W, _N = 16, 1024, 2048, 64
_NCORES = 8

_BUILD_CACHE = {}

_BIG = 32767


def _register_or_op():
    """Fused DVE op: out = in0 | in1; accum_out = OR-fold(out).

    Lets one pass build the half-ORed plane and its full row-reduction:
    reads 2 elems/cycle via both ports instead of the 1/cycle of a plain
    tensor_reduce over the full plane.
    """
    import numpy as np
    import concourse.dve_ops as dve_ops
    from concourse.dve_spec import Spec, Src0, Src1, Bin, AluOp, Zero, lower
    from concourse.dve_table_gen import dve_ver_for, DveOpSpec

    for o in dve_ops.OPS:
        if o.name == "ANT_OR_OR":
            return o

    def _ref(in0, in1, c0, c1, c2):
        b = np.bitwise_or(in0, in1)
        return b, np.bitwise_or.reduce(
            b.reshape(b.shape[0], -1), axis=-1, keepdims=True)

    body = Bin(AluOp.BITWISE_OR, Src0, Src1)
    spec = Spec(body=body, accum=AluOp.BITWISE_OR, accum_init=Zero,
                reference=_ref)
    opcode = dve_ops._CUSTOM_DVE_ROW_BASE + len(dve_ops.OPS)
    shas = {}
    for ver in ("v3", "v4"):
        try:
            shas[ver] = DveOpSpec(name="ANT_OR_OR", opcode=opcode,
                                  uops=lower(spec, ver=ver),
                                  rd1_en=True).sha(ver)
        except Exception:
            pass
    op = dve_ops.DveOp("ANT_OR_OR", spec, subdim=False, uops_sha=shas)
    dve_ops.OPS.append(op)
    dve_ops.CUSTOM_DVE_SPECS[op.name] = spec
    dve_ops._SUB_OPCODE_FOR_NAME[op.name] = opcode
    return op


def _build_tables(TL, H, W):
    P = 128
    CH = H // P
    p = np.arange(P)
    ch = np.arange(CH)
    y = (128 * ch[None, :] + p[:, None]).astype(np.int64)       # [P, CH]
    t = np.arange(32)
    x = (64 * t[None, :] + (p[:, None] // 2)).astype(np.int64)  # [P, 32]
    j32 = np.tile(np.arange(32), 2)                             # [64]
    return {
        "ymBn": (_BIG - y).astype(np.int32),
        "yp1": (y + 1).astype(np.int32),
        "xmBn": (_BIG - x).astype(np.int16),
        "xp1": (x + 1).astype(np.int16),
        "sp32": np.broadcast_to(j32, (P, 64)).astype(np.int32).copy(),
        "sp16": np.broadcast_to(np.arange(16), (P, 16)).astype(np.int16).copy(),
    }


def _build_program(TL, H, W, reps=1, dbg=False):
    from contextlib import ExitStack

    import concourse.bass as bass
    import concourse.tile as tile
    import concourse.mybir as mybir
    from concourse.alu_op_type import AluOpType as Op

    f32 = mybir.dt.float32
    i32 = mybir.dt.int32
    u32 = mybir.dt.uint32
    u16 = mybir.dt.uint16
    i16 = mybir.dt.int16
    Copy = mybir.ActivationFunctionType.Copy
    X = mybir.AxisListType.X

    P = 128
    CH = H // P                 # 8 row chunks per frame
    NT = 32                     # u16-word blocks per plane after transpose
    FC = TL * CH

    tables = _build_tables(TL, H, W)

    nc = bass.Bass()
    seg_in = nc.dram_tensor("seg", [TL, H, W], i32, kind="ExternalInput")
    boxes_out = nc.dram_tensor("boxes", [TL, 64, 4], f32, kind="ExternalOutput")

    def _dt(a):
        return {np.dtype(np.int32): i32, np.dtype(np.int16): i16,
                np.dtype(np.float32): f32}[a.dtype]

    d_tab = {
        n: nc.dram_tensor(n, list(tables[n].shape), _dt(tables[n]),
                          kind="ExternalInput")
        for n in tables
    }

    if dbg:
        dbg_rmask = nc.dram_tensor("dbg_rmask", [P, 2, TL, CH], i32,
                                   kind="ExternalOutput")
        dbg_colm = nc.dram_tensor("dbg_colm", [P, TL, 2, NT], i16,
                                  kind="ExternalOutput")
        dbg_S = nc.dram_tensor("dbg_S", [P, 384], i16, kind="ExternalOutput")
        dbg_ST = nc.dram_tensor("dbg_ST", [P, 3, 128], i16,
                                kind="ExternalOutput")
        dbg_xst = nc.dram_tensor("dbg_xst", [P, 32, 2], i16,
                                 kind="ExternalOutput")

    with tile.TileContext(nc) as tc, ExitStack() as ctx:
        constp = ctx.enter_context(tc.tile_pool(name="consts", bufs=1))
        segp = ctx.enter_context(tc.tile_pool(name="segp", bufs=3))
        ep = ctx.enter_context(tc.tile_pool(name="ep", bufs=3))
        accp = ctx.enter_context(tc.tile_pool(name="accp", bufs=2))
        accTp = ctx.enter_context(tc.tile_pool(name="accTp", bufs=2))
        scrp = ctx.enter_context(tc.tile_pool(name="scrp", bufs=2))
        maskp = ctx.enter_context(tc.tile_pool(name="maskp", bufs=1))
        xp = ctx.enter_context(tc.tile_pool(name="xp", bufs=1))
        smallp = ctx.enter_context(tc.tile_pool(name="smallp", bufs=1))

        c_tab = {}
        for n in tables:
            c_tab[n] = constp.tile([P, tables[n].shape[1]], _dt(tables[n]),
                                   name=f"tab_{n}")
            nc.sync.dma_start(c_tab[n][:], d_tab[n][:])

        for _rep in range(reps):
            rmask = maskp.tile([P, 2, TL, CH], i32, tag="rmask")
            colm = maskp.tile([P, TL, 2, NT], i16, tag="colm")

            # ================= main loop =================
            for f in range(TL):
                acc = accp.tile([P, 2, W], i32, tag="acc")
                for c in range(CH):
                    s = segp.tile([P, W], i32, tag="seg")
                    for k in range(2):
                        nc.sync.dma_start(
                            s[64 * k:64 * (k + 1), :],
                            seg_in[f, c * P + 64 * k:c * P + 64 * (k + 1), :])

                    tgt = acc if c == 0 else ep.tile([P, 2, W], i32, tag="e")
                    # lo plane: int bits of fp32 2^(31-s) = (158-s)<<23
                    if c % 2 == 0:
                        nc.scalar.activation(tgt[:, 0, :], s[:], Copy,
                                             bias=1325400064.0,
                                             scale=-8388608.0)
                    else:
                        nc.gpsimd.tensor_scalar(tgt[:, 0, :], s[:], -8388608,
                                                1325400064, Op.mult, Op.add)
                    # hi plane: int bits of fp32 2^(s-32) = (95+s)<<23
                    nc.gpsimd.tensor_scalar(tgt[:, 1, :], s[:], 8388608,
                                            796917760, Op.mult, Op.add)
                    # per-plane: truncating cast f32 -> u32, or-accumulate,
                    # and row-mask OR-reduce (plane 0 work starts while
                    # plane 1 still casts)
                    for pl in range(2):
                        nc.scalar.activation(tgt[:, pl, :].bitcast(u32),
                                             tgt[:, pl, :].bitcast(f32), Copy)
                        if c > 0:
                            nc.vector.tensor_tensor(acc[:, pl, :].bitcast(u32),
                                                    tgt[:, pl, :].bitcast(u32),
                                                    acc[:, pl, :].bitcast(u32),
                                                    Op.bitwise_or)
                        nc.vector.tensor_reduce(
                            rmask[:, pl, f, c:c + 1].bitcast(u32),
                            tgt[:, pl, :].bitcast(u32), axis=X,
                            op=Op.bitwise_or)

                # ---- column masks: transpose acc (u16) + OR over rows ----
                accT = accTp.tile([P, 2, NT, 128], u16, tag="accT")
                for pl in range(2):
                    a16 = acc[:, pl, :].bitcast(u16)   # [P, 4096]
                    for k in range(4):
                        nc.sync.dma_start(accT[:, pl, 8 * k:8 * (k + 1), :],
                                          a16[:, 1024 * k:1024 * (k + 1)],
                                          transpose=True)
                aT = accT[:].bitcast(u32)              # [P, 2, NT, 64]
                redT = smallp.tile([P, 2, NT], u32, tag="redT")
                nc.vector.tensor_reduce(redT[:].unsqueeze(3), aT[:], axis=X,
                                        op=Op.bitwise_or)
                # merge u16 halves (even/odd source rows), keep low u16
                hv = smallp.tile([P, 2, NT], u32, tag="hv")
                nc.vector.tensor_scalar(hv[:], redT[:], 16, None,
                                        Op.logical_shift_right)
                nc.vector.tensor_tensor(hv[:], hv[:], redT[:],
                                        Op.bitwise_or)
                hv16 = hv[:].bitcast(u16).rearrange(
                    "p a (b two) -> p a b two", two=2)
                nc.vector.tensor_copy(colm[:, f, :, :].bitcast(u16),
                                      hv16[:, :, :, 0])

            # ================= extraction =================
            if dbg:
                nc.sync.dma_start(dbg_rmask[:], rmask[:])
                nc.sync.dma_start(dbg_colm[:], colm[:])

            S = smallp.tile([P, 3, 128], i16, tag="S")

            # ---- row side: E = (rmask >> sp) & 1, dims [pl, sp, (f ch)] ----
            E32 = xp.tile([P, 2, 32, TL, CH], i32, tag="E32")
            rm_b = (rmask[:].rearrange("p a b c -> p a (b c)")
                    .unsqueeze(2).broadcast_to([P, 2, 32, FC]))
            sp_b = (c_tab["sp32"][:].rearrange("p (a b) -> p a b", a=2)
                    .unsqueeze(3).broadcast_to([P, 2, 32, FC]))
            E32f = E32[:].rearrange("p a b c d -> p a b (c d)").bitcast(u32)
            nc.vector.tensor_tensor(E32f, rm_b.bitcast(u32), sp_b.bitcast(u32),
                                    Op.logical_shift_right)
            e32flat = E32[:].rearrange("p a b c d -> p (a b c d)").bitcast(u32)
            nc.vector.tensor_scalar(e32flat, e32flat, 1, None, Op.bitwise_and)

            # select values (i16); min side pre-negated -> MAX folds
            vr = xp.tile([P, 2, 64, TL, CH], i16, tag="vr")   # [k, (pl sp), f, ch]
            E32_3 = E32[:].rearrange("p a b c d -> p (a b) c d")
            ymBn_b = (c_tab["ymBn"][:].unsqueeze(1).unsqueeze(1)
                      .broadcast_to([P, 64, TL, CH]))
            yp1_b = (c_tab["yp1"][:].unsqueeze(1).unsqueeze(1)
                     .broadcast_to([P, 64, TL, CH]))
            nc.vector.tensor_tensor(vr[:, 0], E32_3, ymBn_b, Op.mult)
            nc.vector.tensor_tensor(vr[:, 1], E32_3, yp1_b, Op.mult)
            vr0 = vr[:, 0].rearrange("p a b c -> p (a b c)")
            nc.vector.tensor_scalar(vr0, vr0, -_BIG, None, Op.add)

            # fold over ch (8 -> 1): 3 MAX levels, both k at once
            vrf = vr[:].rearrange("p k a b c -> p (k a b) c")
            w = CH // 2
            while w >= 1:
                nc.vector.tensor_tensor(vrf[:, :, 0:w], vrf[:, :, 0:w],
                                        vrf[:, :, w:2 * w], Op.max)
                w //= 2
            # S rows 0/1: col idx = (pl sp)*TL + f = n*TL + f
            nc.vector.tensor_copy(
                S[:, 0, :], vr[:, 0].rearrange("p a b c -> p (a b) c")[:, :, 0])
            nc.vector.tensor_copy(
                S[:, 1, :], vr[:, 1].rearrange("p a b c -> p (a b) c")[:, :, 0])

            # ---- col side: E = (colm >> sp16) & 1, dims [(f pl), sp16, t] --
            E16 = xp.tile([P, TL * 2, 16, NT], i16, tag="E16")
            cm_b = (colm[:].rearrange("p a b c -> p (a b) c")
                    .unsqueeze(2).broadcast_to([P, TL * 2, 16, NT]))
            s16_b = (c_tab["sp16"][:].unsqueeze(1).unsqueeze(3)
                     .broadcast_to([P, TL * 2, 16, NT]))
            nc.vector.tensor_tensor(E16[:].bitcast(u16), cm_b.bitcast(u16),
                                    s16_b.bitcast(u16),
                                    Op.logical_shift_right)
            e16flat = E16[:].rearrange("p a b c -> p (a b c)").bitcast(u16)
            nc.vector.tensor_scalar(e16flat, e16flat, 1, None, Op.bitwise_and)

            vc = xp.tile([P, 2, TL * 2 * 16, NT], i16, tag="vc")
            E16_3 = E16[:].rearrange("p a b c -> p (a b) c")
            xmBn_b = (c_tab["xmBn"][:].unsqueeze(1)
                      .broadcast_to([P, TL * 2 * 16, NT]))
            xp1_b = (c_tab["xp1"][:].unsqueeze(1)
                     .broadcast_to([P, TL * 2 * 16, NT]))
            nc.vector.tensor_tensor(vc[:, 0], E16_3, xmBn_b, Op.mult)
            nc.vector.tensor_tensor(vc[:, 1], E16_3, xp1_b, Op.mult)
            vc0 = vc[:, 0].rearrange("p a b -> p (a b)")
            nc.vector.tensor_scalar(vc0, vc0, -_BIG, None, Op.add)

            # fold over t (32 -> 1): 5 MAX levels, both k at once
            vcf = vc[:].rearrange("p k a b -> p (k a) b")
            w = NT // 2
            while w >= 1:
                nc.vector.tensor_tensor(vcf[:, :, 0:w], vcf[:, :, 0:w],
                                        vcf[:, :, w:2 * w], Op.max)
                w //= 2
            # S row 2: [-xmin stats | xmax stats], idx = f*32 + pl*16 + sp16
            nc.vector.tensor_copy(S[:, 2, 0:64], vc[:, 0, :, 0])
            nc.vector.tensor_copy(S[:, 2, 64:128], vc[:, 1, :, 0])

            if dbg:
                nc.sync.dma_start(dbg_S[:],
                                  S[:].rearrange("p a b -> p (a b)"))

            # ---- partition fold via one u16 transpose ----
            ST = smallp.tile([P, 3, 128], i16, tag="ST")
            nc.sync.dma_start(ST[:], S[:].rearrange("p a b -> p (a b)"),
                              transpose=True)
            # rows 0/1: fold over all 128 source partitions
            yst = smallp.tile([P, 2, 1], i16, tag="yst")
            nc.vector.tensor_reduce(yst[:], ST[:, 0:2, :], axis=X, op=Op.max)
            # row 2: parity-separated fold (64 pairs) -> [q, h]
            ST2 = ST[:, 2, :].rearrange("p (a two) -> p a two", two=2)
            xst = smallp.tile([P, 32, 2], i16, tag="xst")
            nc.vector.tensor_tensor(xst[:], ST2[:, 0:32, :], ST2[:, 32:64, :],
                                    Op.max)
            w = 16
            while w >= 1:
                nc.vector.tensor_tensor(xst[:, 0:w, :], xst[:, 0:w, :],
                                        xst[:, w:2 * w, :], Op.max)
                w //= 2
            # xst[q, 0, h]: q = k*64 + (f*32 + pl*16 + sp16)
            if dbg:
                nc.sync.dma_start(dbg_ST[:], ST[:])
                nc.sync.dma_start(dbg_xst[:], xst[:])

            # ---- finalize in-layout; output DMAs do the permutation ----
            # row side: yst[p = n*TL + f, {0: -ymin, 1: ymax+1}]
            YF = smallp.tile([P, 2], f32, tag="YF")
            yfix = smallp.tile([P, 2], f32, tag="yfix")
            nc.vector.tensor_copy(YF[:], yst[:, :, 0])
            nc.vector.tensor_scalar(YF[:, 0:1], YF[:, 0:1], -1.0, 0.0,
                                    Op.mult, Op.add)
            nc.vector.tensor_scalar(YF[:, 1:2], YF[:, 1:2], 1.0, 0.0,
                                    Op.subtract, Op.add)
            nc.vector.tensor_scalar(yfix[:, 0:1], YF[:, 0:1], 32767.0,
                                    2147450880.0, Op.is_equal, Op.mult)
            nc.vector.tensor_scalar(yfix[:, 1:2], YF[:, 1:2], -1.0,
                                    -2147483647.0, Op.is_equal, Op.mult)
            nc.vector.tensor_tensor(YF[:], YF[:], yfix[:], Op.add)
            # boxes[f, n, k] for k in {1, 3} <- YF[n*TL + f, k2]
            ydst = boxes_out[:].rearrange("f n (k2 two) -> n f k2 two",
                                          k2=2, two=2)[:, :, :, 1]
            ysrc = YF[:].rearrange("(n f) k -> n f k", n=64, f=TL)
            for fi in range(TL):
                nc.sync.dma_start(ydst[:, fi], ysrc[:, fi])

            # col side: xst[q = k*64 + f*32 + pl*16 + sp, 0, h]
            XF = smallp.tile([P, 2], f32, tag="XF")
            xfix = smallp.tile([P, 2], f32, tag="xfix")
            nc.vector.tensor_copy(XF[:], xst[:, 0, :])
            nc.vector.tensor_scalar(XF[0:64, :], XF[0:64, :], -1.0, 0.0,
                                    Op.mult, Op.add)
            nc.vector.tensor_scalar(XF[64:128, :], XF[64:128, :], 1.0, 0.0,
                                    Op.subtract, Op.add)
            nc.vector.tensor_scalar(xfix[0:64, :], XF[0:64, :], 32767.0,
                                    2147450880.0, Op.is_equal, Op.mult)
            nc.vector.tensor_scalar(xfix[64:128, :], XF[64:128, :], -1.0,
                                    -2147483647.0, Op.is_equal, Op.mult)
            nc.vector.tensor_tensor(XF[:], XF[:], xfix[:], Op.add)
            # boxes[f, 32*pl+16*h+sp, k] for k in {0, 2}
            xdst_all = boxes_out[:].rearrange(
                "f (pl h sp) k -> k f pl sp h", pl=2, h=2, sp=16)
            for k in range(2):
                for fi in range(TL):
                    base = 64 * k + 32 * fi
                    src_ap = XF[base:base + 32, :].rearrange(
                        "(pl sp) h -> pl sp h", pl=2, sp=16)
                    for pl in range(2):
                        nc.sync.dma_start(xdst_all[2 * k, fi, pl],
                                          src_ap[pl])

    nc.finalize()
    _split_excess_waits(nc, mybir)
    return nc, tables


def _split_excess_waits(nc, mybir):
    """Hoist extra sem waits onto preceding NoOps (walrus encoding limit)."""
    n_split = 0
    for f in nc.m.functions:
        for bb in f.blocks:
            newl = []
            for ins in bb.instructions:
                si = ins.sync_info
                max_waits = 1
                if si and si.on_wait and len(si.on_wait) > max_waits:
                    waits = list(si.on_wait)
                    for j, w in enumerate(waits[max_waits:]):
                        nop = mybir.InstNoOp(
                            name=f"{ins.name}-w{j}", ins=[], outs=[],
                            engine=ins.engine,
                            sync_info=mybir.SyncInfo(on_wait=[w],
                                                     on_update=[]))
                        newl.append(nop)
                        n_split += 1
                    ins.sync_info = mybir.SyncInfo(on_wait=waits[:max_waits],
                                                   on_update=si.on_update)
                newl.append(ins)
            bb.instructions = newl
    return n_split


def _get_program(TL, H, W, reps=1, dbg=False):
    key = (TL, H, W, reps, dbg)
    if key not in _BUILD_CACHE:
        _BUILD_CACHE[key] = _build_program(TL, H, W, reps=reps, dbg=dbg)
    return _BUILD_CACHE[key]


def kernel(segmentation, num_instances=None, **_ignored):
    from concourse.bass_utils import run_bass_kernel_spmd

    seg = np.asarray(segmentation)
    T, H, W = seg.shape
    assert T % _NCORES == 0
    TL = T // _NCORES
    nc, tables = _get_program(TL, H, W)

    seg = np.ascontiguousarray(seg, dtype=np.int32)
    in_maps = [{"seg": seg[i * TL:(i + 1) * TL], **tables}
               for i in range(_NCORES)]
    res = run_bass_kernel_spmd(nc, in_maps, list(range(_NCORES)))
    out = np.concatenate([res.results[i]["boxes"] for i in range(_NCORES)],
                         axis=0)
    return out.astype(np.float32)
